# revision 35
# baseline (speedup 1.0000x reference)
import math
import numpy as np

B, C, T, NH, HD = 16, 512, 1024, 8, 64
NCORES = 8
BPC = B // NCORES
CT = C // 128
ST = T // 128
TH = T // 512
VH = BPC * NH
NU = VH // 2
EPS = 1e-5
EXP_SHIFT = -2.0
VW = HD + 2

_CACHE = {}


def _build_nc():
    import concourse.bass as bass
    from concourse import bacc
    import concourse.tile as tile
    from concourse import mybir
    from contextlib import ExitStack

    F32 = mybir.dt.float32
    FP8 = mybir.dt.float8e4
    AF = mybir.ActivationFunctionType
    OP = mybir.AluOpType
    DR = mybir.MatmulPerfMode.DoubleRow

    nc = bacc.Bacc(trn_type="TRN2", name="attn6")

    x = nc.dram_tensor("x", [BPC, C, T], F32, kind="ExternalInput")
    wqk = nc.dram_tensor("wqk", [128, 2, 2, 2 * C], FP8, kind="ExternalInput")
    wv = nc.dram_tensor("wv", [128, 2, 2, C], FP8, kind="ExternalInput")
    wp = nc.dram_tensor("wp", [128, 2, 2, C], FP8, kind="ExternalInput")
    bqk = nc.dram_tensor("bqk", [128, NH], F32, kind="ExternalInput")
    pb = nc.dram_tensor("pb", [128, CT], F32, kind="ExternalInput")
    nw = nc.dram_tensor("nw", [128, CT], F32, kind="ExternalInput")
    nb = nc.dram_tensor("nb", [128, CT], F32, kind="ExternalInput")
    em = nc.dram_tensor("em", [8, 128], F32, kind="ExternalInput")
    gm = nc.dram_tensor("gm", [128, 8], F32, kind="ExternalInput")
    vones = nc.dram_tensor("vones", [128, 2, NH, 2], FP8, kind="ExternalInput")
    y = nc.dram_tensor("y", [BPC, C, T], F32, kind="ExternalOutput")

    with tile.TileContext(nc) as tc, ExitStack() as ctx:
        consts = ctx.enter_context(tc.tile_pool(name="consts", bufs=1))
        xpool = ctx.enter_context(tc.tile_pool(name="xpool", bufs=2))
        hpool = ctx.enter_context(tc.tile_pool(name="hpool", bufs=2))
        qppool = ctx.enter_context(tc.tile_pool(name="qppool", bufs=3))
        ppool = ctx.enter_context(tc.tile_pool(name="ppool", bufs=4))
        pint = ctx.enter_context(tc.tile_pool(name="pint", bufs=2))
        vpool = ctx.enter_context(tc.tile_pool(name="vpool", bufs=8))
        apool = ctx.enter_context(tc.tile_pool(name="apool", bufs=2))
        opool = ctx.enter_context(tc.tile_pool(name="opool", bufs=2))
        ypool = ctx.enter_context(tc.tile_pool(name="ypool", bufs=4))
        zpool = ctx.enter_context(tc.tile_pool(name="zpool", bufs=4))
        rspool = ctx.enter_context(tc.tile_pool(name="rspool", bufs=2))
        rbpool = ctx.enter_context(tc.tile_pool(name="rbpool", bufs=2))
        tmp = ctx.enter_context(tc.tile_pool(name="tmp", bufs=4))
        psS = ctx.enter_context(tc.tile_pool(name="psS", bufs=2, space="PSUM"))
        psO = ctx.enter_context(tc.tile_pool(name="psO", bufs=2, space="PSUM"))
        psQ = psO

        xs_pre = []
        for b in range(BPC):
            x_s = xpool.tile([128, CT, T], F32, tag="x", name=f"x{b}")
            xr_ = x.ap()[b].rearrange("(j p) t -> p j t", p=128)
            for j in range(CT):
                nc.sync.dma_start(out=x_s[:, j, :], in_=xr_[:, j, :])
            xs_pre.append(x_s)

        wqk_s = consts.tile([128, 2, 2, 2 * C], FP8)
        nc.sync.dma_start(out=wqk_s, in_=wqk.ap())
        wv_s = consts.tile([128, 2, 2, C], FP8)
        nc.sync.dma_start(out=wv_s, in_=wv.ap())
        wp_s = consts.tile([128, 2, 2, C], FP8)
        nc.sync.dma_start(out=wp_s, in_=wp.ap())
        bqk_s = consts.tile([128, NH], F32)
        nc.sync.dma_start(out=bqk_s, in_=bqk.ap())
        pb_s = consts.tile([128, CT], F32)
        nc.sync.dma_start(out=pb_s, in_=pb.ap())
        nw_s = consts.tile([128, CT], F32)
        nc.sync.dma_start(out=nw_s, in_=nw.ap())
        nb_s = consts.tile([128, CT], F32)
        nc.sync.dma_start(out=nb_s, in_=nb.ap())
        em_s = consts.tile([8, 128], F32)
        nc.sync.dma_start(out=em_s, in_=em.ap())
        gm_s = consts.tile([128, 8], F32)
        nc.sync.dma_start(out=gm_s, in_=gm.ap())
        eps_s = consts.tile([8, 1], F32)
        nc.vector.memset(eps_s, EPS)
        neg2 = consts.tile([128, 1], F32)
        nc.vector.memset(neg2, EXP_SHIFT)
        kblk = [consts.tile([128, 16, 128], FP8, name=f"kblk{i}")
                for i in range(2)]
        nc.vector.memset(kblk[0], 0.0)
        nc.vector.memset(kblk[1], 0.0)

        x_t = [None] * BPC
        h_t = [None] * BPC
        qp_t = [None] * NU
        Pi_t = [None] * NU
        P_t = [None] * VH
        v_t = [[None] * (ST // 2) for _ in range(BPC)]
        a_t = [None] * BPC
        o_t = [None] * VH

        def unit_heads(u):
            hp, b = divmod(u, 2)
            return b, (2 * hp, 2 * hp + 1)

        def emit_groupnorm(b):
            x_s = xs_pre[b]
            x_t[b] = x_s
            s2_all = tmp.tile([128, 8], F32, tag="s2")
            for j in range(CT):
                st = tmp.tile([128, 2, 6], F32, tag="st")
                nc.vector.bn_stats(out=st[:, 0, :], in_=x_s[:, j, 0:512])
                nc.vector.bn_stats(out=st[:, 1, :], in_=x_s[:, j, 512:1024])
                mv = tmp.tile([128, 2], F32, tag="mv")
                nc.vector.bn_aggr(out=mv, in_=st)
                nc.vector.tensor_copy(out=s2_all[:, j:j + 1], in_=mv[:, 0:1])
                nc.vector.scalar_tensor_tensor(
                    out=s2_all[:, 4 + j:5 + j], in0=mv[:, 0:1],
                    scalar=mv[:, 0:1], in1=mv[:, 1:2],
                    op0=OP.mult, op1=OP.add,
                )
            gs = psQ.tile([8, 8], F32, tag="Q")
            nc.tensor.matmul(gs, gm_s, s2_all, start=True, stop=True)
            gsb = tmp.tile([8, 8], F32, tag="gsb")
            nc.vector.tensor_copy(out=gsb, in_=gs)
            msq = tmp.tile([8, 4], F32, tag="msq")
            nc.vector.tensor_mul(out=msq, in0=gsb[:, 0:4], in1=gsb[:, 0:4])
            varg = tmp.tile([8, 4], F32, tag="varg")
            nc.vector.tensor_tensor(out=varg, in0=gsb[:, 4:8], in1=msq,
                                    op=OP.subtract)
            lng = tmp.tile([8, 4], F32, tag="lng")
            nc.scalar.activation(out=lng, in_=varg, func=AF.Ln, bias=eps_s)
            rstd = tmp.tile([8, 4], F32, tag="rstd")
            nc.scalar.activation(out=rstd, in_=lng, func=AF.Exp, scale=-0.5)
            mr = tmp.tile([8, 8], F32, tag="mr")
            nc.vector.tensor_copy(out=mr[:, 0:4], in_=gsb[:, 0:4])
            nc.vector.tensor_copy(out=mr[:, 4:8], in_=rstd)
            mexp = psQ.tile([128, 8], F32, tag="Q")
            nc.tensor.matmul(mexp, em_s, mr, start=True, stop=True)
            scale_c = tmp.tile([128, CT], F32, tag="scale_c")
            nc.vector.tensor_mul(out=scale_c, in0=mexp[:, 4:8], in1=nw_s)
            mscl = tmp.tile([128, CT], F32, tag="mscl")
            nc.vector.tensor_mul(out=mscl, in0=mexp[:, 0:4], in1=scale_c)
            bias_c = tmp.tile([128, CT], F32, tag="bias_c")
            nc.vector.tensor_tensor(out=bias_c, in0=nb_s, in1=mscl,
                                    op=OP.subtract)
            h_s = hpool.tile([128, CT, T], FP8, tag="h")
            for j in range(CT):
                nc.vector.tensor_scalar(
                    out=h_s[:, j, :], in0=x_s[:, j, :],
                    scalar1=scale_c[:, j:j + 1], scalar2=bias_c[:, j:j + 1],
                    op0=OP.mult, op1=OP.add,
                )
            h_t[b] = h_s

        def emit_qk(u, jj, th):
            b, heads = unit_heads(u)
            j = heads[jj]
            h_s = h_t[b]
            pq = psQ.tile([128, 512], F32, tag="Q", name=f"pq{u}_{jj}_{th}")
            for p in range(2):
                nc.tensor.matmul(
                    pq,
                    wqk_s[:, p, :, j * 128:(j + 1) * 128],
                    h_s[:, 2 * p:2 * p + 2, th * 512:(th + 1) * 512],
                    start=(p == 0), stop=(p == 1), perf_mode=DR,
                )
            if jj == 0 and th == 0:
                qp_t[u] = qppool.tile([128, T], FP8, tag="qp", name=f"qp{u}")
            sl = slice(th * 512, (th + 1) * 512)
            nc.vector.tensor_scalar_add(
                out=qp_t[u][64 * jj:64 * jj + 64, sl], in0=pq[0:64, :],
                scalar1=bqk_s[0:64, j:j + 1])
            kout = kblk[u % 2][64 * jj:64 * jj + 64, 8 * th:8 * th + 8,
                               64 * jj:64 * jj + 64]
            if u == 0:
                nc.scalar.copy(out=kout, in_=pq[64:128, :])
            else:
                nc.vector.tensor_copy(out=kout, in_=pq[64:128, :])

        def emit_v(b, i):
            h_s = h_t[b]
            pv = psQ.tile([128, 512], F32, tag="Q", name=f"pv{b}_{i}")
            for p in range(2):
                nc.tensor.matmul(
                    pv,
                    h_s[:, 2 * p:2 * p + 2, i * 128:(i + 1) * 128],
                    wv_s[:, p, :, :],
                    start=(p == 0), stop=(p == 1), perf_mode=DR,
                )
            if i % 2 == 0:
                v_t[b][i // 2] = vpool.tile(
                    [128, 2, NH, VW], FP8, tag="v", name=f"v{b}_{i // 2}")
                nc.sync.dma_start(out=v_t[b][i // 2][:, :, :, HD:HD + 2],
                                  in_=vones.ap())
            nc.vector.tensor_copy(
                out=v_t[b][i // 2][:, i % 2, :, 0:HD],
                in_=pv.rearrange("p (h d) -> p h d", d=HD),
            )

        def emit_S(u, m):
            pS = psS.tile([128, T], F32, tag="S", name=f"pS{u}_{m}")
            for th in range(TH):
                nc.tensor.matmul(
                    pS[:, th * 512:(th + 1) * 512],
                    kblk[u % 2][:, m, :],
                    qp_t[u][:, th * 512:(th + 1) * 512],
                    start=True, stop=True,
                )
            return pS

        def emit_exp(u, m, pS):
            if m == 0:
                Pi_t[u] = pint.tile([128, 16, T], FP8, tag="Pi",
                                    name=f"Pi{u}")
            pos = (m % 2) * 8 + (m // 4) * 2 + (m // 2) % 2
            nc.scalar.activation(out=Pi_t[u][:, pos, :], in_=pS,
                                 func=AF.Exp, scale=0.125, bias=neg2)

        def emit_pdma(u, half):
            b, heads = unit_heads(u)
            if half == 0:
                for jj in (0, 1):
                    P_t[heads[jj] * 2 + b] = ppool.tile(
                        [128, ST // 2, 2, T], FP8, tag="P",
                        name=f"P{heads[jj] * 2 + b}")
            for jj in (0, 1):
                vh = heads[jj] * 2 + b
                nc.sync.dma_start(
                    out=P_t[vh][64 * half:64 * half + 64, :, :, :],
                    in_=Pi_t[u][64 * jj:64 * jj + 64,
                                8 * half:8 * half + 8, :])

        def emit_PV(vh, th, copy_dve=False):
            j, b = divmod(vh, 2)
            if th == 0:
                o_t[vh] = opool.tile([HD + 2, T], F32, tag="o", name=f"o{vh}")
            pO = psO.tile([HD + 2, 512], F32, tag="O", name=f"pO{vh}_{th}")
            for p in range(ST // 2):
                nc.tensor.matmul(
                    pO,
                    v_t[b][p][:, :, j, :],
                    P_t[vh][:, p, :, th * 512:(th + 1) * 512],
                    start=(p == 0), stop=(p == ST // 2 - 1), perf_mode=DR,
                )
            nc.vector.tensor_copy(
                out=o_t[vh][:, th * 512:(th + 1) * 512], in_=pO)

        def emit_chain(vh, fast=False):
            j, b = divmod(vh, 2)
            o_sb = o_t[vh]
            r_s = rspool.tile([1, T], F32, tag="r")
            if fast:
                nc.vector.reciprocal(out=r_s, in_=o_sb[HD:HD + 1, :])
            else:
                zres = zpool.tile([128, T // 128], F32, tag="zres")
                nc.sync.dma_start(out=zres, in_=o_sb[HD:HD + 1, :])
                zrec = zpool.tile([128, T // 128], F32, tag="zrec")
                nc.vector.reciprocal(out=zrec, in_=zres)
                nc.sync.dma_start(out=r_s, in_=zrec)
            rb_s = rbpool.tile([64, T], F32, tag="rb")
            nc.gpsimd.partition_broadcast(out_ap=rb_s, in_ap=r_s)
            if a_t[b] is None:
                a_t[b] = apool.tile([128, 2, 2, T], FP8, tag="a",
                                    name=f"a{b}")
            po2 = (j % 2) * 64
            nc.vector.tensor_mul(
                out=a_t[b][po2:po2 + 64, j // 4, (j // 2) % 2, :],
                in0=o_sb[0:HD, :], in1=rb_s,
            )

        def emit_proj(b, jo, th):
            pp = psQ.tile([128, 512], F32, tag="Q", name=f"pp{b}_{jo}_{th}")
            for p in range(2):
                nc.tensor.matmul(
                    pp,
                    wp_s[:, p, :, jo * 128:(jo + 1) * 128],
                    a_t[b][:, p, :, th * 512:(th + 1) * 512],
                    start=(p == 0), stop=(p == 1), perf_mode=DR,
                )
            y_s = ypool.tile([128, 512], F32, tag="y")
            sl = slice(th * 512, (th + 1) * 512)
            nc.vector.scalar_tensor_tensor(
                out=y_s, in0=pp, scalar=pb_s[:, jo:jo + 1],
                in1=x_t[b][:, jo, sl], op0=OP.add, op1=OP.add,
            )
            nc.sync.dma_start(
                out=y.ap()[b, 128 * jo:128 * (jo + 1), sl], in_=y_s)

        emit_groupnorm(0)
        for jj in (0, 1):
            for th in (0, 1):
                emit_qk(0, jj, th)
        emit_groupnorm(1)
        for jj in (0, 1):
            for th in (0, 1):
                emit_qk(1, jj, th)

        vunits = [(b, i) for b in range(BPC) for i in range(ST)]
        vinj = {}
        for n, (vb, vi) in enumerate(vunits):
            slot = 2 + n // 2 if n < 12 else 8 + (n - 12)
            vinj.setdefault((0, slot), []).append((vb, vi))

        projA = [(0, jo, th) for jo in range(CT) for th in range(TH)]
        projB = [(1, jo, th) for jo in range(CT) for th in range(TH)]

        NSLOT = 16
        for u in range(NU):
            b, heads = unit_heads(u)
            if u >= 1:
                pb_, pheads = unit_heads(u - 1)
                pvhA = pheads[0] * 2 + pb_
                pvhB = pheads[1] * 2 + pb_
            for idx in range(NSLOT):
                m = 2 * idx if idx < 8 else 2 * (idx - 8) + 1
                pS = emit_S(u, m)
                emit_exp(u, m, pS)
                if idx in (7, 15):
                    emit_pdma(u, idx // 8)
                for (vb, vi) in vinj.get((u, idx), []):
                    emit_v(vb, vi)
                if 1 <= u < NU - 1 and idx in (1, 3, 5, 7):
                    qi = (idx - 1) // 2
                    emit_qk(u + 1, qi // 2, qi % 2)
                if u >= 1:
                    if idx == 4:
                        emit_PV(pvhA, 0)
                    elif idx == 6:
                        emit_PV(pvhA, 1)
                    elif idx == 8:
                        emit_PV(pvhB, 0)
                    elif idx == 10:
                        emit_PV(pvhB, 1)
                    elif idx == 11:
                        emit_chain(pvhA)
                    elif idx == 12:
                        emit_chain(pvhB)
                if u == NU - 1 and 13 <= idx <= 15:
                    emit_proj(*projA[2 * (idx - 13)])
                    emit_proj(*projA[2 * (idx - 13) + 1])
                if u == NU - 1 and idx == 15:
                    emit_proj(*projA[6])
                    emit_proj(*projA[7])
        lb, lheads = unit_heads(NU - 1)
        for vh in (lheads[0] * 2 + lb, lheads[1] * 2 + lb):
            emit_PV(vh, 0, copy_dve=True)
            emit_PV(vh, 1, copy_dve=True)
            emit_chain(vh)
        for pu in projB:
            emit_proj(*pu)

    nc.finalize()
    return nc


def _prepack(qkv_w, qkv_b, proj_w, proj_b, norm_w, norm_b):
    import ml_dtypes

    def to_fp8_tiles(w, ncols):
        wr = w.reshape(2, 2, 128, ncols).transpose(2, 0, 1, 3)
        wr = np.clip(wr, -240.0, 240.0)
        return np.ascontiguousarray(wr).astype(ml_dtypes.float8_e4m3fn)

    wqk = np.empty((C, 2 * C), dtype=np.float32)
    bqk = np.empty((128, NH), dtype=np.float32)
    wv = np.empty((C, C), dtype=np.float32)
    bv = np.empty((C,), dtype=np.float32)
    for h in range(NH):
        base = 3 * HD * h
        wqk[:, 128 * h:128 * h + HD] = qkv_w[base:base + HD, :].T
        wqk[:, 128 * h + HD:128 * h + 128] = qkv_w[base + HD:base + 128, :].T
        bqk[:, h] = qkv_b[base:base + 128]
        wv[:, HD * h:HD * (h + 1)] = qkv_w[base + 128:base + 192, :].T
        bv[HD * h:HD * (h + 1)] = qkv_b[base + 128:base + 192]
    wp = np.ascontiguousarray(proj_w.T)
    pbv = proj_b + proj_w @ bv
    pb = np.ascontiguousarray(pbv.reshape(CT, 128).T)
    nw = np.ascontiguousarray(norm_w.reshape(CT, 128).T)
    nb = np.ascontiguousarray(norm_b.reshape(CT, 128).T)
    em = np.zeros((8, 128), dtype=np.float32)
    gm = np.zeros((128, 8), dtype=np.float32)
    for p in range(128):
        em[p // 16, p] = 1.0
        gm[p, p // 16] = 1.0 / 16.0
    vones = np.ones((128, 2, NH, 2), dtype=ml_dtypes.float8_e4m3fn)
    vones[:, :, :, 1] = 0.0
    return dict(
        wqk=to_fp8_tiles(wqk, 2 * C), bqk=bqk,
        wv=to_fp8_tiles(wv, C), wp=to_fp8_tiles(wp, C),
        pb=pb, nw=nw, nb=nb, em=em, gm=gm, vones=vones,
    )


def kernel(**inputs):
    from concourse.bass_utils import run_bass_kernel_spmd

    x = np.ascontiguousarray(np.asarray(inputs["x"], dtype=np.float32))
    assert x.shape == (B, C, 32, 32)
    nh = int(np.asarray(inputs["num_heads"]))
    assert nh == NH, f"kernel hardcodes num_heads={NH}, got {nh}"

    packed = _prepack(
        np.asarray(inputs["qkv_w"], dtype=np.float32),
        np.asarray(inputs["qkv_b"], dtype=np.float32),
        np.asarray(inputs["proj_w"], dtype=np.float32),
        np.asarray(inputs["proj_b"], dtype=np.float32),
        np.asarray(inputs["norm_w"], dtype=np.float32),
        np.asarray(inputs["norm_b"], dtype=np.float32),
    )

    if "nc" not in _CACHE:
        _CACHE["nc"] = _build_nc()
    nc = _CACHE["nc"]

    xr = x.reshape(B, C, T)
    in_maps = []
    for c in range(NCORES):
        m = dict(packed)
        m["x"] = np.ascontiguousarray(xr[c * BPC:(c + 1) * BPC])
        in_maps.append(m)

    def run_once():
        res = run_bass_kernel_spmd(nc, in_maps, core_ids=list(range(NCORES)))
        return np.concatenate(
            [res.results[c]["y"] for c in range(NCORES)], axis=0
        )

    out1 = run_once()
    out2 = run_once()
    if not np.array_equal(out1, out2):
        out3 = run_once()
        out1 = out3 if np.array_equal(out2, out3) else out2
        if np.array_equal(out2, out3):
            out1 = out2
    return out1.reshape(B, C, 32, 32).astype(np.float32)


# revision 36
# speedup vs baseline: 1.1741x; 1.1741x over previous
import math
import numpy as np

B, C, T, NH, HD = 16, 512, 1024, 8, 64
NCORES = 8
BPC = B // NCORES
CT = C // 128
ST = T // 128
TH = T // 512
VH = BPC * NH
NU = VH // 2
EPS = 1e-5
EXP_SHIFT = -2.0
VW = HD + 2

_CACHE = {}


def _build_nc():
    import concourse.bass as bass
    from concourse import bacc
    import concourse.tile as tile
    from concourse import mybir
    from contextlib import ExitStack

    F32 = mybir.dt.float32
    FP8 = mybir.dt.float8e4
    AF = mybir.ActivationFunctionType
    OP = mybir.AluOpType
    DR = mybir.MatmulPerfMode.DoubleRow

    nc = bacc.Bacc(trn_type="TRN2", name="attn6")

    x = nc.dram_tensor("x", [BPC, C, T], F32, kind="ExternalInput")
    wqk = nc.dram_tensor("wqk", [128, 2, 2, 2 * C], FP8, kind="ExternalInput")
    wv = nc.dram_tensor("wv", [128, 2, 2, C], FP8, kind="ExternalInput")
    wp = nc.dram_tensor("wp", [128, 2, 2, C], FP8, kind="ExternalInput")
    bqk = nc.dram_tensor("bqk", [128, NH], F32, kind="ExternalInput")
    pb = nc.dram_tensor("pb", [128, CT], F32, kind="ExternalInput")
    nw = nc.dram_tensor("nw", [128, CT], F32, kind="ExternalInput")
    nb = nc.dram_tensor("nb", [128, CT], F32, kind="ExternalInput")
    em = nc.dram_tensor("em", [8, 128], F32, kind="ExternalInput")
    gm = nc.dram_tensor("gm", [128, 8], F32, kind="ExternalInput")
    vones = nc.dram_tensor("vones", [128, 2, NH, 2], FP8, kind="ExternalInput")
    y = nc.dram_tensor("y", [BPC, C, T], F32, kind="ExternalOutput")

    with tile.TileContext(nc) as tc, ExitStack() as ctx:
        consts = ctx.enter_context(tc.tile_pool(name="consts", bufs=1))
        xpool = ctx.enter_context(tc.tile_pool(name="xpool", bufs=2))
        hpool = ctx.enter_context(tc.tile_pool(name="hpool", bufs=2))
        qppool = ctx.enter_context(tc.tile_pool(name="qppool", bufs=3))
        ppool = ctx.enter_context(tc.tile_pool(name="ppool", bufs=4))
        pint = ctx.enter_context(tc.tile_pool(name="pint", bufs=2))
        vpool = ctx.enter_context(tc.tile_pool(name="vpool", bufs=8))
        apool = ctx.enter_context(tc.tile_pool(name="apool", bufs=2))
        opool = ctx.enter_context(tc.tile_pool(name="opool", bufs=2))
        ypool = ctx.enter_context(tc.tile_pool(name="ypool", bufs=4))
        zpool = ctx.enter_context(tc.tile_pool(name="zpool", bufs=4))
        rspool = ctx.enter_context(tc.tile_pool(name="rspool", bufs=2))
        rbpool = ctx.enter_context(tc.tile_pool(name="rbpool", bufs=2))
        tmp = ctx.enter_context(tc.tile_pool(name="tmp", bufs=4))
        psS = ctx.enter_context(tc.tile_pool(name="psS", bufs=2, space="PSUM"))
        psO = ctx.enter_context(tc.tile_pool(name="psO", bufs=2, space="PSUM"))
        psQ = psO

        wqk_s = consts.tile([128, 2, 2, 2 * C], FP8)
        nc.sync.dma_start(out=wqk_s, in_=wqk.ap())
        wv_s = consts.tile([128, 2, 2, C], FP8)
        nc.sync.dma_start(out=wv_s, in_=wv.ap())
        wp_s = consts.tile([128, 2, 2, C], FP8)
        nc.sync.dma_start(out=wp_s, in_=wp.ap())
        bqk_s = consts.tile([128, NH], F32)
        nc.sync.dma_start(out=bqk_s, in_=bqk.ap())
        pb_s = consts.tile([128, CT], F32)
        nc.sync.dma_start(out=pb_s, in_=pb.ap())
        nw_s = consts.tile([128, CT], F32)
        nc.sync.dma_start(out=nw_s, in_=nw.ap())
        nb_s = consts.tile([128, CT], F32)
        nc.sync.dma_start(out=nb_s, in_=nb.ap())
        em_s = consts.tile([8, 128], F32)
        nc.sync.dma_start(out=em_s, in_=em.ap())
        gm_s = consts.tile([128, 8], F32)
        nc.sync.dma_start(out=gm_s, in_=gm.ap())
        eps_s = consts.tile([8, 1], F32)
        nc.vector.memset(eps_s, EPS)
        neg2 = consts.tile([128, 1], F32)
        nc.vector.memset(neg2, EXP_SHIFT)
        kblk = [consts.tile([128, 16, 128], FP8, name=f"kblk{i}")
                for i in range(2)]
        nc.vector.memset(kblk[0], 0.0)
        nc.vector.memset(kblk[1], 0.0)

        x_t = [None] * BPC
        h_t = [None] * BPC
        qp_t = [None] * NU
        Pi_t = [None] * NU
        P_t = [None] * VH
        v_t = [[None] * (ST // 2) for _ in range(BPC)]
        a_t = [None] * BPC
        o_t = [None] * VH

        def unit_heads(u):
            hp, b = divmod(u, 2)
            return b, (2 * hp, 2 * hp + 1)

        def emit_groupnorm(b):
            x_s = xpool.tile([128, CT, T], F32, tag="x")
            xr = x.ap()[b].rearrange("(j p) t -> p j t", p=128)
            for j in range(CT):
                nc.sync.dma_start(out=x_s[:, j, :], in_=xr[:, j, :])
            x_t[b] = x_s
            s2_all = tmp.tile([128, 8], F32, tag="s2")
            for j in range(CT):
                st = tmp.tile([128, 2, 6], F32, tag="st")
                nc.vector.bn_stats(out=st[:, 0, :], in_=x_s[:, j, 0:512])
                nc.vector.bn_stats(out=st[:, 1, :], in_=x_s[:, j, 512:1024])
                mv = tmp.tile([128, 2], F32, tag="mv")
                nc.vector.bn_aggr(out=mv, in_=st)
                nc.vector.tensor_copy(out=s2_all[:, j:j + 1], in_=mv[:, 0:1])
                nc.vector.scalar_tensor_tensor(
                    out=s2_all[:, 4 + j:5 + j], in0=mv[:, 0:1],
                    scalar=mv[:, 0:1], in1=mv[:, 1:2],
                    op0=OP.mult, op1=OP.add,
                )
            gs = psQ.tile([8, 8], F32, tag="Q")
            nc.tensor.matmul(gs, gm_s, s2_all, start=True, stop=True)
            gsb = tmp.tile([8, 8], F32, tag="gsb")
            nc.vector.tensor_copy(out=gsb, in_=gs)
            msq = tmp.tile([8, 4], F32, tag="msq")
            nc.vector.tensor_mul(out=msq, in0=gsb[:, 0:4], in1=gsb[:, 0:4])
            varg = tmp.tile([8, 4], F32, tag="varg")
            nc.vector.tensor_tensor(out=varg, in0=gsb[:, 4:8], in1=msq,
                                    op=OP.subtract)
            lng = tmp.tile([8, 4], F32, tag="lng")
            nc.scalar.activation(out=lng, in_=varg, func=AF.Ln, bias=eps_s)
            rstd = tmp.tile([8, 4], F32, tag="rstd")
            nc.scalar.activation(out=rstd, in_=lng, func=AF.Exp, scale=-0.5)
            mr = tmp.tile([8, 8], F32, tag="mr")
            nc.vector.tensor_copy(out=mr[:, 0:4], in_=gsb[:, 0:4])
            nc.vector.tensor_copy(out=mr[:, 4:8], in_=rstd)
            mexp = psQ.tile([128, 8], F32, tag="Q")
            nc.tensor.matmul(mexp, em_s, mr, start=True, stop=True)
            scale_c = tmp.tile([128, CT], F32, tag="scale_c")
            nc.vector.tensor_mul(out=scale_c, in0=mexp[:, 4:8], in1=nw_s)
            mscl = tmp.tile([128, CT], F32, tag="mscl")
            nc.vector.tensor_mul(out=mscl, in0=mexp[:, 0:4], in1=scale_c)
            bias_c = tmp.tile([128, CT], F32, tag="bias_c")
            nc.vector.tensor_tensor(out=bias_c, in0=nb_s, in1=mscl,
                                    op=OP.subtract)
            h_s = hpool.tile([128, CT, T], FP8, tag="h")
            for j in range(CT):
                nc.vector.tensor_scalar(
                    out=h_s[:, j, :], in0=x_s[:, j, :],
                    scalar1=scale_c[:, j:j + 1], scalar2=bias_c[:, j:j + 1],
                    op0=OP.mult, op1=OP.add,
                )
            h_t[b] = h_s

        def emit_qk(u, jj, th):
            b, heads = unit_heads(u)
            j = heads[jj]
            h_s = h_t[b]
            pq = psQ.tile([128, 512], F32, tag="Q", name=f"pq{u}_{jj}_{th}")
            for p in range(2):
                nc.tensor.matmul(
                    pq,
                    wqk_s[:, p, :, j * 128:(j + 1) * 128],
                    h_s[:, 2 * p:2 * p + 2, th * 512:(th + 1) * 512],
                    start=(p == 0), stop=(p == 1), perf_mode=DR,
                )
            if jj == 0 and th == 0:
                qp_t[u] = qppool.tile([128, T], FP8, tag="qp", name=f"qp{u}")
            sl = slice(th * 512, (th + 1) * 512)
            nc.vector.tensor_scalar_add(
                out=qp_t[u][64 * jj:64 * jj + 64, sl], in0=pq[0:64, :],
                scalar1=bqk_s[0:64, j:j + 1])
            kout = kblk[u % 2][64 * jj:64 * jj + 64, 8 * th:8 * th + 8,
                               64 * jj:64 * jj + 64]
            if u == 0:
                nc.scalar.copy(out=kout, in_=pq[64:128, :])
            else:
                nc.vector.tensor_copy(out=kout, in_=pq[64:128, :])

        def emit_v(b, i):
            h_s = h_t[b]
            pv = psQ.tile([128, 512], F32, tag="Q", name=f"pv{b}_{i}")
            for p in range(2):
                nc.tensor.matmul(
                    pv,
                    h_s[:, 2 * p:2 * p + 2, i * 128:(i + 1) * 128],
                    wv_s[:, p, :, :],
                    start=(p == 0), stop=(p == 1), perf_mode=DR,
                )
            if i % 2 == 0:
                v_t[b][i // 2] = vpool.tile(
                    [128, 2, NH, VW], FP8, tag="v", name=f"v{b}_{i // 2}")
                nc.sync.dma_start(out=v_t[b][i // 2][:, :, :, HD:HD + 2],
                                  in_=vones.ap())
            nc.vector.tensor_copy(
                out=v_t[b][i // 2][:, i % 2, :, 0:HD],
                in_=pv.rearrange("p (h d) -> p h d", d=HD),
            )

        def emit_S(u, m):
            pS = psS.tile([128, T], F32, tag="S", name=f"pS{u}_{m}")
            for th in range(TH):
                nc.tensor.matmul(
                    pS[:, th * 512:(th + 1) * 512],
                    kblk[u % 2][:, m, :],
                    qp_t[u][:, th * 512:(th + 1) * 512],
                    start=True, stop=True,
                )
            return pS

        def emit_exp(u, m, pS):
            if m == 0:
                Pi_t[u] = pint.tile([128, 16, T], FP8, tag="Pi",
                                    name=f"Pi{u}")
            pos = (m % 2) * 8 + (m // 4) * 2 + (m // 2) % 2
            nc.scalar.activation(out=Pi_t[u][:, pos, :], in_=pS,
                                 func=AF.Exp, scale=0.125, bias=neg2)

        def emit_pdma(u, half):
            b, heads = unit_heads(u)
            if half == 0:
                for jj in (0, 1):
                    P_t[heads[jj] * 2 + b] = ppool.tile(
                        [128, ST // 2, 2, T], FP8, tag="P",
                        name=f"P{heads[jj] * 2 + b}")
            for jj in (0, 1):
                vh = heads[jj] * 2 + b
                nc.sync.dma_start(
                    out=P_t[vh][64 * half:64 * half + 64, :, :, :],
                    in_=Pi_t[u][64 * jj:64 * jj + 64,
                                8 * half:8 * half + 8, :])

        def emit_PV(vh, th, copy_dve=False):
            j, b = divmod(vh, 2)
            if th == 0:
                o_t[vh] = opool.tile([HD + 2, T], F32, tag="o", name=f"o{vh}")
            pO = psO.tile([HD + 2, 512], F32, tag="O", name=f"pO{vh}_{th}")
            for p in range(ST // 2):
                nc.tensor.matmul(
                    pO,
                    v_t[b][p][:, :, j, :],
                    P_t[vh][:, p, :, th * 512:(th + 1) * 512],
                    start=(p == 0), stop=(p == ST // 2 - 1), perf_mode=DR,
                )
            nc.vector.tensor_copy(
                out=o_t[vh][:, th * 512:(th + 1) * 512], in_=pO)

        def emit_chain(vh, fast=False):
            j, b = divmod(vh, 2)
            o_sb = o_t[vh]
            r_s = rspool.tile([1, T], F32, tag="r")
            if fast:
                nc.vector.reciprocal(out=r_s, in_=o_sb[HD:HD + 1, :])
            else:
                zres = zpool.tile([128, T // 128], F32, tag="zres")
                nc.sync.dma_start(out=zres, in_=o_sb[HD:HD + 1, :])
                zrec = zpool.tile([128, T // 128], F32, tag="zrec")
                nc.vector.reciprocal(out=zrec, in_=zres)
                nc.sync.dma_start(out=r_s, in_=zrec)
            rb_s = rbpool.tile([64, T], F32, tag="rb")
            nc.gpsimd.partition_broadcast(out_ap=rb_s, in_ap=r_s)
            if a_t[b] is None:
                a_t[b] = apool.tile([128, 2, 2, T], FP8, tag="a",
                                    name=f"a{b}")
            po2 = (j % 2) * 64
            nc.vector.tensor_mul(
                out=a_t[b][po2:po2 + 64, j // 4, (j // 2) % 2, :],
                in0=o_sb[0:HD, :], in1=rb_s,
            )

        def emit_proj(b, jo, th):
            pp = psQ.tile([128, 512], F32, tag="Q", name=f"pp{b}_{jo}_{th}")
            for p in range(2):
                nc.tensor.matmul(
                    pp,
                    wp_s[:, p, :, jo * 128:(jo + 1) * 128],
                    a_t[b][:, p, :, th * 512:(th + 1) * 512],
                    start=(p == 0), stop=(p == 1), perf_mode=DR,
                )
            y_s = ypool.tile([128, 512], F32, tag="y")
            sl = slice(th * 512, (th + 1) * 512)
            nc.vector.scalar_tensor_tensor(
                out=y_s, in0=pp, scalar=pb_s[:, jo:jo + 1],
                in1=x_t[b][:, jo, sl], op0=OP.add, op1=OP.add,
            )
            nc.sync.dma_start(
                out=y.ap()[b, 128 * jo:128 * (jo + 1), sl], in_=y_s)

        emit_groupnorm(0)
        for jj in (0, 1):
            for th in (0, 1):
                emit_qk(0, jj, th)
        emit_groupnorm(1)
        for jj in (0, 1):
            for th in (0, 1):
                emit_qk(1, jj, th)

        vunits = [(b, i) for b in range(BPC) for i in range(ST)]
        vinj = {}
        for n, (vb, vi) in enumerate(vunits):
            slot = 2 + n // 2 if n < 12 else 8 + (n - 12)
            vinj.setdefault((0, slot), []).append((vb, vi))

        projA = [(0, jo, th) for jo in range(CT) for th in range(TH)]
        projB = [(1, jo, th) for jo in range(CT) for th in range(TH)]

        NSLOT = 16
        for u in range(NU):
            b, heads = unit_heads(u)
            if u >= 1:
                pb_, pheads = unit_heads(u - 1)
                pvhA = pheads[0] * 2 + pb_
                pvhB = pheads[1] * 2 + pb_
            for idx in range(NSLOT):
                m = 2 * idx if idx < 8 else 2 * (idx - 8) + 1
                pS = emit_S(u, m)
                emit_exp(u, m, pS)
                if idx in (7, 15):
                    emit_pdma(u, idx // 8)
                for (vb, vi) in vinj.get((u, idx), []):
                    emit_v(vb, vi)
                if 1 <= u < NU - 1 and idx in (1, 3, 5, 7):
                    qi = (idx - 1) // 2
                    emit_qk(u + 1, qi // 2, qi % 2)
                if u >= 1:
                    if idx == 4:
                        emit_PV(pvhA, 0)
                    elif idx == 6:
                        emit_PV(pvhA, 1)
                    elif idx == 8:
                        emit_PV(pvhB, 0)
                    elif idx == 10:
                        emit_PV(pvhB, 1)
                    elif idx == 11:
                        emit_chain(pvhA)
                    elif idx == 12:
                        emit_chain(pvhB)
                if u == NU - 1 and 13 <= idx <= 15:
                    emit_proj(*projA[2 * (idx - 13)])
                    emit_proj(*projA[2 * (idx - 13) + 1])
                if u == NU - 1 and idx == 15:
                    emit_proj(*projA[6])
                    emit_proj(*projA[7])
        lb, lheads = unit_heads(NU - 1)
        for vh in (lheads[0] * 2 + lb, lheads[1] * 2 + lb):
            emit_PV(vh, 0, copy_dve=True)
            emit_PV(vh, 1, copy_dve=True)
            emit_chain(vh)
        for pu in projB:
            emit_proj(*pu)

    nc.finalize()
    return nc


def _prepack(qkv_w, qkv_b, proj_w, proj_b, norm_w, norm_b):
    import ml_dtypes

    def to_fp8_tiles(w, ncols):
        wr = w.reshape(2, 2, 128, ncols).transpose(2, 0, 1, 3)
        wr = np.clip(wr, -240.0, 240.0)
        return np.ascontiguousarray(wr).astype(ml_dtypes.float8_e4m3fn)

    wqk = np.empty((C, 2 * C), dtype=np.float32)
    bqk = np.empty((128, NH), dtype=np.float32)
    wv = np.empty((C, C), dtype=np.float32)
    bv = np.empty((C,), dtype=np.float32)
    for h in range(NH):
        base = 3 * HD * h
        wqk[:, 128 * h:128 * h + HD] = qkv_w[base:base + HD, :].T
        wqk[:, 128 * h + HD:128 * h + 128] = qkv_w[base + HD:base + 128, :].T
        bqk[:, h] = qkv_b[base:base + 128]
        wv[:, HD * h:HD * (h + 1)] = qkv_w[base + 128:base + 192, :].T
        bv[HD * h:HD * (h + 1)] = qkv_b[base + 128:base + 192]
    wp = np.ascontiguousarray(proj_w.T)
    pbv = proj_b + proj_w @ bv
    pb = np.ascontiguousarray(pbv.reshape(CT, 128).T)
    nw = np.ascontiguousarray(norm_w.reshape(CT, 128).T)
    nb = np.ascontiguousarray(norm_b.reshape(CT, 128).T)
    em = np.zeros((8, 128), dtype=np.float32)
    gm = np.zeros((128, 8), dtype=np.float32)
    for p in range(128):
        em[p // 16, p] = 1.0
        gm[p, p // 16] = 1.0 / 16.0
    vones = np.ones((128, 2, NH, 2), dtype=ml_dtypes.float8_e4m3fn)
    vones[:, :, :, 1] = 0.0
    return dict(
        wqk=to_fp8_tiles(wqk, 2 * C), bqk=bqk,
        wv=to_fp8_tiles(wv, C), wp=to_fp8_tiles(wp, C),
        pb=pb, nw=nw, nb=nb, em=em, gm=gm, vones=vones,
    )


def kernel(**inputs):
    from concourse.bass_utils import run_bass_kernel_spmd

    x = np.ascontiguousarray(np.asarray(inputs["x"], dtype=np.float32))
    assert x.shape == (B, C, 32, 32)
    nh = int(np.asarray(inputs["num_heads"]))
    assert nh == NH, f"kernel hardcodes num_heads={NH}, got {nh}"

    packed = _prepack(
        np.asarray(inputs["qkv_w"], dtype=np.float32),
        np.asarray(inputs["qkv_b"], dtype=np.float32),
        np.asarray(inputs["proj_w"], dtype=np.float32),
        np.asarray(inputs["proj_b"], dtype=np.float32),
        np.asarray(inputs["norm_w"], dtype=np.float32),
        np.asarray(inputs["norm_b"], dtype=np.float32),
    )

    if "nc" not in _CACHE:
        _CACHE["nc"] = _build_nc()
    nc = _CACHE["nc"]

    xr = x.reshape(B, C, T)
    in_maps = []
    for c in range(NCORES):
        m = dict(packed)
        m["x"] = np.ascontiguousarray(xr[c * BPC:(c + 1) * BPC])
        in_maps.append(m)

    def run_once():
        res = run_bass_kernel_spmd(nc, in_maps, core_ids=list(range(NCORES)))
        return np.concatenate(
            [res.results[c]["y"] for c in range(NCORES)], axis=0
        )

    out1 = run_once()
    out2 = run_once()
    if not np.array_equal(out1, out2):
        out3 = run_once()
        out1 = out3 if np.array_equal(out2, out3) else out2
        if np.array_equal(out2, out3):
            out1 = out2
    return out1.reshape(B, C, 32, 32).astype(np.float32)


# revision 37
# speedup vs baseline: 1.1758x; 1.0014x over previous
import math
import numpy as np

B, C, T, NH, HD = 16, 512, 1024, 8, 64
NCORES = 8
BPC = B // NCORES
CT = C // 128
ST = T // 128
TH = T // 512
VH = BPC * NH
NU = VH // 2
EPS = 1e-5
EXP_SHIFT = -2.0
VW = HD + 2

_CACHE = {}


def _build_nc():
    import concourse.bass as bass
    from concourse import bacc
    import concourse.tile as tile
    from concourse import mybir
    from contextlib import ExitStack

    F32 = mybir.dt.float32
    FP8 = mybir.dt.float8e4
    AF = mybir.ActivationFunctionType
    OP = mybir.AluOpType
    DR = mybir.MatmulPerfMode.DoubleRow

    nc = bacc.Bacc(trn_type="TRN2", name="attn6")

    x = nc.dram_tensor("x", [BPC, C, T], F32, kind="ExternalInput")
    wqk = nc.dram_tensor("wqk", [128, 2, 2, 2 * C], FP8, kind="ExternalInput")
    wv = nc.dram_tensor("wv", [128, 2, 2, C], FP8, kind="ExternalInput")
    wp = nc.dram_tensor("wp", [128, 2, 2, C], FP8, kind="ExternalInput")
    bqk = nc.dram_tensor("bqk", [128, NH], F32, kind="ExternalInput")
    pb = nc.dram_tensor("pb", [128, CT], F32, kind="ExternalInput")
    nw = nc.dram_tensor("nw", [128, CT], F32, kind="ExternalInput")
    nb = nc.dram_tensor("nb", [128, CT], F32, kind="ExternalInput")
    em = nc.dram_tensor("em", [8, 128], F32, kind="ExternalInput")
    gm = nc.dram_tensor("gm", [128, 8], F32, kind="ExternalInput")
    vones = nc.dram_tensor("vones", [128, 2, NH, 2], FP8, kind="ExternalInput")
    y = nc.dram_tensor("y", [BPC, C, T], F32, kind="ExternalOutput")

    with tile.TileContext(nc) as tc, ExitStack() as ctx:
        consts = ctx.enter_context(tc.tile_pool(name="consts", bufs=1))
        xpool = ctx.enter_context(tc.tile_pool(name="xpool", bufs=2))
        hpool = ctx.enter_context(tc.tile_pool(name="hpool", bufs=2))
        qppool = ctx.enter_context(tc.tile_pool(name="qppool", bufs=3))
        ppool = ctx.enter_context(tc.tile_pool(name="ppool", bufs=4))
        pint = ctx.enter_context(tc.tile_pool(name="pint", bufs=2))
        vpool = ctx.enter_context(tc.tile_pool(name="vpool", bufs=8))
        apool = ctx.enter_context(tc.tile_pool(name="apool", bufs=2))
        opool = ctx.enter_context(tc.tile_pool(name="opool", bufs=2))
        ypool = ctx.enter_context(tc.tile_pool(name="ypool", bufs=4))
        zpool = ctx.enter_context(tc.tile_pool(name="zpool", bufs=4))
        rspool = ctx.enter_context(tc.tile_pool(name="rspool", bufs=2))
        rbpool = ctx.enter_context(tc.tile_pool(name="rbpool", bufs=2))
        tmp = ctx.enter_context(tc.tile_pool(name="tmp", bufs=4))
        psS = ctx.enter_context(tc.tile_pool(name="psS", bufs=2, space="PSUM"))
        psO = ctx.enter_context(tc.tile_pool(name="psO", bufs=2, space="PSUM"))
        psQ = psO

        wqk_s = consts.tile([128, 2, 2, 2 * C], FP8)
        nc.sync.dma_start(out=wqk_s, in_=wqk.ap())
        wv_s = consts.tile([128, 2, 2, C], FP8)
        nc.sync.dma_start(out=wv_s, in_=wv.ap())
        wp_s = consts.tile([128, 2, 2, C], FP8)
        nc.sync.dma_start(out=wp_s, in_=wp.ap())
        bqk_s = consts.tile([128, NH], F32)
        nc.sync.dma_start(out=bqk_s, in_=bqk.ap())
        pb_s = consts.tile([128, CT], F32)
        nc.sync.dma_start(out=pb_s, in_=pb.ap())
        nw_s = consts.tile([128, CT], F32)
        nc.sync.dma_start(out=nw_s, in_=nw.ap())
        nb_s = consts.tile([128, CT], F32)
        nc.sync.dma_start(out=nb_s, in_=nb.ap())
        em_s = consts.tile([8, 128], F32)
        nc.sync.dma_start(out=em_s, in_=em.ap())
        gm_s = consts.tile([128, 8], F32)
        nc.sync.dma_start(out=gm_s, in_=gm.ap())
        eps_s = consts.tile([8, 1], F32)
        nc.vector.memset(eps_s, EPS)
        neg2 = consts.tile([128, 1], F32)
        nc.vector.memset(neg2, EXP_SHIFT)
        kblk = [consts.tile([128, 16, 128], FP8, name=f"kblk{i}")
                for i in range(2)]
        nc.vector.memset(kblk[0], 0.0)
        nc.vector.memset(kblk[1], 0.0)

        x_t = [None] * BPC
        h_t = [None] * BPC
        qp_t = [None] * NU
        Pi_t = [None] * NU
        P_t = [None] * VH
        v_t = [[None] * (ST // 2) for _ in range(BPC)]
        a_t = [None] * BPC
        o_t = [None] * VH

        def unit_heads(u):
            hp, b = divmod(u, 2)
            return b, (2 * hp, 2 * hp + 1)

        def emit_groupnorm(b):
            x_s = xpool.tile([128, CT, T], F32, tag="x")
            xr = x.ap()[b].rearrange("(j p) t -> p j t", p=128)
            for j in range(CT):
                nc.sync.dma_start(out=x_s[:, j, :], in_=xr[:, j, :])
            x_t[b] = x_s
            s2_all = tmp.tile([128, 8], F32, tag="s2")
            for j in range(CT):
                st = tmp.tile([128, 2, 6], F32, tag="st")
                nc.vector.bn_stats(out=st[:, 0, :], in_=x_s[:, j, 0:512])
                nc.vector.bn_stats(out=st[:, 1, :], in_=x_s[:, j, 512:1024])
                mv = tmp.tile([128, 2], F32, tag="mv")
                nc.vector.bn_aggr(out=mv, in_=st)
                nc.vector.tensor_copy(out=s2_all[:, j:j + 1], in_=mv[:, 0:1])
                nc.vector.scalar_tensor_tensor(
                    out=s2_all[:, 4 + j:5 + j], in0=mv[:, 0:1],
                    scalar=mv[:, 0:1], in1=mv[:, 1:2],
                    op0=OP.mult, op1=OP.add,
                )
            gs = psQ.tile([8, 8], F32, tag="Q")
            nc.tensor.matmul(gs, gm_s, s2_all, start=True, stop=True)
            gsb = tmp.tile([8, 8], F32, tag="gsb")
            nc.vector.tensor_copy(out=gsb, in_=gs)
            msq = tmp.tile([8, 4], F32, tag="msq")
            nc.vector.tensor_mul(out=msq, in0=gsb[:, 0:4], in1=gsb[:, 0:4])
            varg = tmp.tile([8, 4], F32, tag="varg")
            nc.vector.tensor_tensor(out=varg, in0=gsb[:, 4:8], in1=msq,
                                    op=OP.subtract)
            lng = tmp.tile([8, 4], F32, tag="lng")
            nc.scalar.activation(out=lng, in_=varg, func=AF.Ln, bias=eps_s)
            rstd = tmp.tile([8, 4], F32, tag="rstd")
            nc.scalar.activation(out=rstd, in_=lng, func=AF.Exp, scale=-0.5)
            mr = tmp.tile([8, 8], F32, tag="mr")
            nc.vector.tensor_copy(out=mr[:, 0:4], in_=gsb[:, 0:4])
            nc.vector.tensor_copy(out=mr[:, 4:8], in_=rstd)
            mexp = psQ.tile([128, 8], F32, tag="Q")
            nc.tensor.matmul(mexp, em_s, mr, start=True, stop=True)
            scale_c = tmp.tile([128, CT], F32, tag="scale_c")
            nc.vector.tensor_mul(out=scale_c, in0=mexp[:, 4:8], in1=nw_s)
            mscl = tmp.tile([128, CT], F32, tag="mscl")
            nc.vector.tensor_mul(out=mscl, in0=mexp[:, 0:4], in1=scale_c)
            bias_c = tmp.tile([128, CT], F32, tag="bias_c")
            nc.vector.tensor_tensor(out=bias_c, in0=nb_s, in1=mscl,
                                    op=OP.subtract)
            h_s = hpool.tile([128, CT, T], FP8, tag="h")
            for j in range(CT):
                nc.vector.tensor_scalar(
                    out=h_s[:, j, :], in0=x_s[:, j, :],
                    scalar1=scale_c[:, j:j + 1], scalar2=bias_c[:, j:j + 1],
                    op0=OP.mult, op1=OP.add,
                )
            h_t[b] = h_s

        def emit_qk(u, jj, th):
            b, heads = unit_heads(u)
            j = heads[jj]
            h_s = h_t[b]
            pq = psQ.tile([128, 512], F32, tag="Q", name=f"pq{u}_{jj}_{th}")
            for p in range(2):
                nc.tensor.matmul(
                    pq,
                    wqk_s[:, p, :, j * 128:(j + 1) * 128],
                    h_s[:, 2 * p:2 * p + 2, th * 512:(th + 1) * 512],
                    start=(p == 0), stop=(p == 1), perf_mode=DR,
                )
            if jj == 0 and th == 0:
                qp_t[u] = qppool.tile([128, T], FP8, tag="qp", name=f"qp{u}")
            sl = slice(th * 512, (th + 1) * 512)
            nc.vector.tensor_scalar_add(
                out=qp_t[u][64 * jj:64 * jj + 64, sl], in0=pq[0:64, :],
                scalar1=bqk_s[0:64, j:j + 1])
            kout = kblk[u % 2][64 * jj:64 * jj + 64, 8 * th:8 * th + 8,
                               64 * jj:64 * jj + 64]
            if u == 0:
                nc.scalar.copy(out=kout, in_=pq[64:128, :])
            else:
                nc.vector.tensor_copy(out=kout, in_=pq[64:128, :])

        def emit_v(b, i):
            h_s = h_t[b]
            pv = psQ.tile([128, 512], F32, tag="Q", name=f"pv{b}_{i}")
            for p in range(2):
                nc.tensor.matmul(
                    pv,
                    h_s[:, 2 * p:2 * p + 2, i * 128:(i + 1) * 128],
                    wv_s[:, p, :, :],
                    start=(p == 0), stop=(p == 1), perf_mode=DR,
                )
            if i % 2 == 0:
                v_t[b][i // 2] = vpool.tile(
                    [128, 2, NH, VW], FP8, tag="v", name=f"v{b}_{i // 2}")
                nc.sync.dma_start(out=v_t[b][i // 2][:, :, :, HD:HD + 2],
                                  in_=vones.ap())
            nc.vector.tensor_copy(
                out=v_t[b][i // 2][:, i % 2, :, 0:HD],
                in_=pv.rearrange("p (h d) -> p h d", d=HD),
            )

        def emit_S(u, m):
            pS = psS.tile([128, T], F32, tag="S", name=f"pS{u}_{m}")
            for th in range(TH):
                nc.tensor.matmul(
                    pS[:, th * 512:(th + 1) * 512],
                    kblk[u % 2][:, m, :],
                    qp_t[u][:, th * 512:(th + 1) * 512],
                    start=True, stop=True,
                )
            return pS

        def emit_exp(u, m, pS):
            if m == 0:
                Pi_t[u] = pint.tile([128, 16, T], FP8, tag="Pi",
                                    name=f"Pi{u}")
            pos = (m % 2) * 8 + (m // 4) * 2 + (m // 2) % 2
            nc.scalar.activation(out=Pi_t[u][:, pos, :], in_=pS,
                                 func=AF.Exp, scale=0.125, bias=neg2)

        def emit_pdma(u, half):
            b, heads = unit_heads(u)
            if half == 0:
                for jj in (0, 1):
                    P_t[heads[jj] * 2 + b] = ppool.tile(
                        [128, ST // 2, 2, T], FP8, tag="P",
                        name=f"P{heads[jj] * 2 + b}")
            for jj in (0, 1):
                vh = heads[jj] * 2 + b
                nc.sync.dma_start(
                    out=P_t[vh][64 * half:64 * half + 64, :, :, :],
                    in_=Pi_t[u][64 * jj:64 * jj + 64,
                                8 * half:8 * half + 8, :])

        def emit_PV(vh, th, copy_dve=False):
            j, b = divmod(vh, 2)
            if th == 0:
                o_t[vh] = opool.tile([HD + 2, T], F32, tag="o", name=f"o{vh}")
            pO = psO.tile([HD + 2, 512], F32, tag="O", name=f"pO{vh}_{th}")
            for p in range(ST // 2):
                nc.tensor.matmul(
                    pO,
                    v_t[b][p][:, :, j, :],
                    P_t[vh][:, p, :, th * 512:(th + 1) * 512],
                    start=(p == 0), stop=(p == ST // 2 - 1), perf_mode=DR,
                )
            nc.vector.tensor_copy(
                out=o_t[vh][:, th * 512:(th + 1) * 512], in_=pO)

        def emit_chain(vh, fast=False):
            j, b = divmod(vh, 2)
            o_sb = o_t[vh]
            r_s = rspool.tile([1, T], F32, tag="r")
            if fast:
                nc.vector.reciprocal(out=r_s, in_=o_sb[HD:HD + 1, :])
            else:
                zres = zpool.tile([128, T // 128], F32, tag="zres")
                nc.sync.dma_start(out=zres, in_=o_sb[HD:HD + 1, :])
                zrec = zpool.tile([128, T // 128], F32, tag="zrec")
                nc.vector.reciprocal(out=zrec, in_=zres)
                nc.sync.dma_start(out=r_s, in_=zrec)
            rb_s = rbpool.tile([64, T], F32, tag="rb")
            nc.gpsimd.partition_broadcast(out_ap=rb_s, in_ap=r_s)
            if a_t[b] is None:
                a_t[b] = apool.tile([128, 2, 2, T], FP8, tag="a",
                                    name=f"a{b}")
            po2 = (j % 2) * 64
            nc.vector.tensor_mul(
                out=a_t[b][po2:po2 + 64, j // 4, (j // 2) % 2, :],
                in0=o_sb[0:HD, :], in1=rb_s,
            )

        def emit_proj(b, jo, th):
            pp = psQ.tile([128, 512], F32, tag="Q", name=f"pp{b}_{jo}_{th}")
            for p in range(2):
                nc.tensor.matmul(
                    pp,
                    wp_s[:, p, :, jo * 128:(jo + 1) * 128],
                    a_t[b][:, p, :, th * 512:(th + 1) * 512],
                    start=(p == 0), stop=(p == 1), perf_mode=DR,
                )
            y_s = ypool.tile([128, 512], F32, tag="y")
            sl = slice(th * 512, (th + 1) * 512)
            nc.vector.scalar_tensor_tensor(
                out=y_s, in0=pp, scalar=pb_s[:, jo:jo + 1],
                in1=x_t[b][:, jo, sl], op0=OP.add, op1=OP.add,
            )
            nc.sync.dma_start(
                out=y.ap()[b, 128 * jo:128 * (jo + 1), sl], in_=y_s)

        emit_groupnorm(0)
        for jj in (0, 1):
            for th in (0, 1):
                emit_qk(0, jj, th)
        emit_groupnorm(1)
        for jj in (0, 1):
            for th in (0, 1):
                emit_qk(1, jj, th)

        vunits = [(b, i) for b in range(BPC) for i in range(ST)]
        vinj = {}
        for n, (vb, vi) in enumerate(vunits):
            slot = 2 + n // 2 if n < 12 else 8 + (n - 12)
            vinj.setdefault((0, slot), []).append((vb, vi))

        projA = [(0, jo, th) for jo in range(CT) for th in range(TH)]
        projB = [(1, jo, th) for jo in range(CT) for th in range(TH)]

        NSLOT = 16
        for u in range(NU):
            b, heads = unit_heads(u)
            if u >= 1:
                pb_, pheads = unit_heads(u - 1)
                pvhA = pheads[0] * 2 + pb_
                pvhB = pheads[1] * 2 + pb_
            for idx in range(NSLOT):
                m = 2 * idx if idx < 8 else 2 * (idx - 8) + 1
                pS = emit_S(u, m)
                emit_exp(u, m, pS)
                if idx in (7, 15):
                    emit_pdma(u, idx // 8)
                for (vb, vi) in vinj.get((u, idx), []):
                    emit_v(vb, vi)
                if 1 <= u < NU - 1 and idx in (1, 3, 5, 7):
                    qi = (idx - 1) // 2
                    emit_qk(u + 1, qi // 2, qi % 2)
                if u >= 1:
                    if idx == 4:
                        emit_PV(pvhA, 0)
                    elif idx == 6:
                        emit_PV(pvhA, 1)
                    elif idx == 8:
                        emit_PV(pvhB, 0)
                    elif idx == 11:
                        emit_chain(pvhA)
                    elif idx == 12:
                        emit_PV(pvhB, 1)
                    elif idx == 13:
                        emit_chain(pvhB)
                if u == NU - 1 and 14 <= idx <= 15:
                    emit_proj(*projA[2 * (idx - 14)])
                    emit_proj(*projA[2 * (idx - 14) + 1])
                if u == NU - 1 and idx == 15:
                    emit_proj(*projA[4])
                    emit_proj(*projA[5])
                    emit_proj(*projA[6])
                    emit_proj(*projA[7])
        lb, lheads = unit_heads(NU - 1)
        for vh in (lheads[0] * 2 + lb, lheads[1] * 2 + lb):
            emit_PV(vh, 0, copy_dve=True)
            emit_PV(vh, 1, copy_dve=True)
            emit_chain(vh)
        for pu in projB:
            emit_proj(*pu)

    nc.finalize()
    return nc


def _prepack(qkv_w, qkv_b, proj_w, proj_b, norm_w, norm_b):
    import ml_dtypes

    def to_fp8_tiles(w, ncols):
        wr = w.reshape(2, 2, 128, ncols).transpose(2, 0, 1, 3)
        wr = np.clip(wr, -240.0, 240.0)
        return np.ascontiguousarray(wr).astype(ml_dtypes.float8_e4m3fn)

    wqk = np.empty((C, 2 * C), dtype=np.float32)
    bqk = np.empty((128, NH), dtype=np.float32)
    wv = np.empty((C, C), dtype=np.float32)
    bv = np.empty((C,), dtype=np.float32)
    for h in range(NH):
        base = 3 * HD * h
        wqk[:, 128 * h:128 * h + HD] = qkv_w[base:base + HD, :].T
        wqk[:, 128 * h + HD:128 * h + 128] = qkv_w[base + HD:base + 128, :].T
        bqk[:, h] = qkv_b[base:base + 128]
        wv[:, HD * h:HD * (h + 1)] = qkv_w[base + 128:base + 192, :].T
        bv[HD * h:HD * (h + 1)] = qkv_b[base + 128:base + 192]
    wp = np.ascontiguousarray(proj_w.T)
    pbv = proj_b + proj_w @ bv
    pb = np.ascontiguousarray(pbv.reshape(CT, 128).T)
    nw = np.ascontiguousarray(norm_w.reshape(CT, 128).T)
    nb = np.ascontiguousarray(norm_b.reshape(CT, 128).T)
    em = np.zeros((8, 128), dtype=np.float32)
    gm = np.zeros((128, 8), dtype=np.float32)
    for p in range(128):
        em[p // 16, p] = 1.0
        gm[p, p // 16] = 1.0 / 16.0
    vones = np.ones((128, 2, NH, 2), dtype=ml_dtypes.float8_e4m3fn)
    vones[:, :, :, 1] = 0.0
    return dict(
        wqk=to_fp8_tiles(wqk, 2 * C), bqk=bqk,
        wv=to_fp8_tiles(wv, C), wp=to_fp8_tiles(wp, C),
        pb=pb, nw=nw, nb=nb, em=em, gm=gm, vones=vones,
    )


def kernel(**inputs):
    from concourse.bass_utils import run_bass_kernel_spmd

    x = np.ascontiguousarray(np.asarray(inputs["x"], dtype=np.float32))
    assert x.shape == (B, C, 32, 32)
    nh = int(np.asarray(inputs["num_heads"]))
    assert nh == NH, f"kernel hardcodes num_heads={NH}, got {nh}"

    packed = _prepack(
        np.asarray(inputs["qkv_w"], dtype=np.float32),
        np.asarray(inputs["qkv_b"], dtype=np.float32),
        np.asarray(inputs["proj_w"], dtype=np.float32),
        np.asarray(inputs["proj_b"], dtype=np.float32),
        np.asarray(inputs["norm_w"], dtype=np.float32),
        np.asarray(inputs["norm_b"], dtype=np.float32),
    )

    if "nc" not in _CACHE:
        _CACHE["nc"] = _build_nc()
    nc = _CACHE["nc"]

    xr = x.reshape(B, C, T)
    in_maps = []
    for c in range(NCORES):
        m = dict(packed)
        m["x"] = np.ascontiguousarray(xr[c * BPC:(c + 1) * BPC])
        in_maps.append(m)

    def run_once():
        res = run_bass_kernel_spmd(nc, in_maps, core_ids=list(range(NCORES)))
        return np.concatenate(
            [res.results[c]["y"] for c in range(NCORES)], axis=0
        )

    out1 = run_once()
    out2 = run_once()
    if not np.array_equal(out1, out2):
        out3 = run_once()
        out1 = out3 if np.array_equal(out2, out3) else out2
        if np.array_equal(out2, out3):
            out1 = out2
    return out1.reshape(B, C, 32, 32).astype(np.float32)
